# revision 4
# baseline (speedup 1.0000x reference)
"""Trainium2 Bass kernel for nn_Encoder_85942295593405 (GNN message passing).

Math (reference):
  emb  = spikes @ W_emb + b_emb                      [b,t,N,D]
  send = relu(relu(emb@Ws1+bs1)@Ws2+bs2)             [b,t,N,D]
  recv = relu(relu(emb@Wr1+br1)@Wr2+br2)             [b,t,N,D]
  full = [send[:,1:,se]|pe[1:]|recv[:,:-1,re]|pe[:-1]]   [b,t-1,E,288]
  out  = relu(full@Wc1+bc1)@Wc2 + bc2                [b,t-1,E,5]

Key factorization: the edge gather commutes with the (linear) first combine
layer, so compute Xs = send@Wc1[0:128,:], Xr = recv@Wc1[144:272,:] at NODE
level (N=128 rows instead of E=1024), then gather to edges via one-hot
gather-matmuls accumulated in PSUM (gather + add fused on the PE), apply the
t-dependent positional-encoding contribution as a per-partition ACT bias
fused with the ReLU, and finish with the thin @Wc2 matmul (bc2 folded in as
a K=1 rank-1 matmul against a ones row).

Sharding: 8 cores = 2 batches x 4 time chunks. Each core computes 64
output timesteps (chunk starts [0,64,128,191]; the last chunk overlaps the
third by one step so all cores run an identical program).
"""

import os
import sys

import numpy as np

sys.path.insert(0, "/opt/trn_rl_repo")

import concourse.bass as bass  # noqa: E402
import concourse.mybir as mybir  # noqa: E402
import concourse.tile as tile  # noqa: E402
from concourse.bass_utils import run_bass_kernel_spmd  # noqa: E402

B, T, N, F = 2, 256, 128, 16
D, H, E = 128, 288, 1024
PE_SIZE = 16
NCORES = 8
TCHUNK = 64            # output timesteps per core
TSTEPS = TCHUNK + 1    # node-level timesteps per core
ROWS = TSTEPS * N      # node-level rows per core (8320)
T_LOS = [0, 64, 128, 191]
FC = 96                # feature chunk for the 288-wide edge activations
NFC = 3                # number of feature chunks (3*96 = 288)
EC = 512               # edge chunk (moving-operand max for fp32)
NEC = E // EC

F32 = mybir.dt.float32

LAST_RESULTS = None    # BassKernelResults of the last run (for test harness)

_PROGRAM = None


def _build_program():
    nc = bass.Bass()

    def inp(name, shape):
        return nc.dram_tensor(name, shape, F32, kind="ExternalInput")

    spk = inp("spk", [F, ROWS])          # spikes, feature-major, per-core slice
    w_emb = inp("w_emb", [F, D])
    b_emb = inp("b_emb", [1, D])         # row (K=1 matmul trick, no relu on emb)
    ws1 = inp("ws1", [D, D])
    bs1 = inp("bs1", [D, 1])             # column (per-partition ACT bias)
    ws2 = inp("ws2", [D, D])
    bs2 = inp("bs2", [D, 1])
    wr1 = inp("wr1", [D, D])
    br1 = inp("br1", [D, 1])
    wr2 = inp("wr2", [D, D])
    br2 = inp("br2", [D, 1])
    wc1s = inp("wc1s", [D, H])           # Wc1[0:128, :]
    wc1r = inp("wc1r", [D, H])           # Wc1[144:272, :]
    gs = inp("gs", [N, E])               # one-hot send gather matrix
    gr = inp("gr", [N, E])               # one-hot recv gather matrix
    pet3 = inp("pet3", [FC, NFC * TCHUNK])  # pe@Wc1 slices + bc1, [96, fc*64+i]
    wc2 = inp("wc2", [FC, NFC, 5])       # Wc2 reshaped to K-chunks
    bc2r = inp("bc2r", [1, 5])
    ones = inp("ones", [1, EC])

    outd = nc.dram_tensor("out", [TCHUNK, 5, E], F32, kind="ExternalOutput")

    relu = mybir.ActivationFunctionType.Relu

    with tile.TileContext(nc) as tc:
        with tc.tile_pool(name="wpool", bufs=1) as wp:
            # persistent weights/constants in SBUF
            w_emb_sb = wp.tile([F, D], F32, tag="w_emb")
            b_emb_sb = wp.tile([1, D], F32, tag="b_emb")
            ws1_sb = wp.tile([D, D], F32, tag="ws1")
            bs1_sb = wp.tile([D, 1], F32, tag="bs1")
            ws2_sb = wp.tile([D, D], F32, tag="ws2")
            bs2_sb = wp.tile([D, 1], F32, tag="bs2")
            wr1_sb = wp.tile([D, D], F32, tag="wr1")
            br1_sb = wp.tile([D, 1], F32, tag="br1")
            wr2_sb = wp.tile([D, D], F32, tag="wr2")
            br2_sb = wp.tile([D, 1], F32, tag="br2")
            wc1s_sb = wp.tile([D, H], F32, tag="wc1s")
            wc1r_sb = wp.tile([D, H], F32, tag="wc1r")
            gs_sb = wp.tile([N, E], F32, tag="gs")
            gr_sb = wp.tile([N, E], F32, tag="gr")
            pet3_sb = wp.tile([FC, NFC * TCHUNK], F32, tag="pet3")
            wc2_sb = wp.tile([FC, NFC, 5], F32, tag="wc2")
            bc2r_sb = wp.tile([1, 5], F32, tag="bc2r")
            ones_sb = wp.tile([1, EC], F32, tag="ones")
            sendT = wp.tile([D, ROWS], F32, tag="sendT")   # node tables,
            recvT = wp.tile([D, ROWS], F32, tag="recvT")   # feature-major

            for sb_t, dr_t in [
                (w_emb_sb, w_emb), (b_emb_sb, b_emb),
                (ws1_sb, ws1), (bs1_sb, bs1), (ws2_sb, ws2), (bs2_sb, bs2),
                (wr1_sb, wr1), (br1_sb, br1), (wr2_sb, wr2), (br2_sb, br2),
                (wc1s_sb, wc1s), (wc1r_sb, wc1r), (gs_sb, gs), (gr_sb, gr),
                (pet3_sb, pet3), (wc2_sb, wc2), (bc2r_sb, bc2r),
                (ones_sb, ones),
            ]:
                nc.sync.dma_start(sb_t[:], dr_t[:])

            # ---- Stage A: node-level MLPs, feature-major ----
            # sendT/recvT[f, k*128+n] = send/recv branch value at local step k,
            # node n, feature f.
            chunks = []
            r0 = 0
            while r0 < ROWS:
                ch = min(512, ROWS - r0)
                chunks.append((r0, ch))
                r0 += ch

            with (
                tc.tile_pool(name="stA_ps", bufs=1, space="PSUM") as psA,
                tc.tile_pool(name="stA_sb", bufs=3) as sbA,
            ):
                for r0, ch in chunks:
                    spk_c = sbA.tile([F, ch], F32, tag="spk")
                    nc.sync.dma_start(spk_c[:], spk[:, r0:r0 + ch])

                    emb_ps = psA.tile([D, ch], F32, tag="emb_ps")
                    nc.tensor.matmul(emb_ps[:], w_emb_sb[:], spk_c[:],
                                     start=True, stop=False)
                    nc.tensor.matmul(emb_ps[:], b_emb_sb[:],
                                     ones_sb[:, :ch], start=False, stop=True)
                    emb_sb = sbA.tile([D, ch], F32, tag="emb_sb")
                    nc.vector.tensor_copy(emb_sb[:], emb_ps[:])

                    # send branch
                    s1_ps = psA.tile([D, ch], F32, tag="s1_ps")
                    nc.tensor.matmul(s1_ps[:], ws1_sb[:], emb_sb[:])
                    s1_sb = sbA.tile([D, ch], F32, tag="s1_sb")
                    nc.scalar.activation(s1_sb[:], s1_ps[:], relu,
                                         bias=bs1_sb[:, 0:1])
                    s2_ps = psA.tile([D, ch], F32, tag="s2_ps")
                    nc.tensor.matmul(s2_ps[:], ws2_sb[:], s1_sb[:])
                    nc.scalar.activation(sendT[:, r0:r0 + ch], s2_ps[:], relu,
                                         bias=bs2_sb[:, 0:1])

                    # recv branch
                    r1_ps = psA.tile([D, ch], F32, tag="r1_ps")
                    nc.tensor.matmul(r1_ps[:], wr1_sb[:], emb_sb[:])
                    r1_sb = sbA.tile([D, ch], F32, tag="r1_sb")
                    nc.scalar.activation(r1_sb[:], r1_ps[:], relu,
                                         bias=br1_sb[:, 0:1])
                    r2_ps = psA.tile([D, ch], F32, tag="r2_ps")
                    nc.tensor.matmul(r2_ps[:], wr2_sb[:], r1_sb[:])
                    nc.scalar.activation(recvT[:, r0:r0 + ch], r2_ps[:], relu,
                                         bias=br2_sb[:, 0:1])

            # ---- Stage B: per output timestep ----
            with (
                tc.tile_pool(name="node_ps", bufs=2, space="PSUM") as nps,
                tc.tile_pool(name="pre_ps", bufs=2, space="PSUM") as pps,
                tc.tile_pool(name="stB_sb", bufs=3) as sbB,
            ):
                ops = nps  # share the 1-bank slots: xs/xr/o_ps all tag "x_ps"
                for i in range(TCHUNK):
                    # Xs at local step i+1 (sender at t+1), Xr at step i.
                    xs_ps = nps.tile([N, H], F32, tag="x_ps")
                    nc.tensor.matmul(
                        xs_ps[:], sendT[:, (i + 1) * N:(i + 2) * N], wc1s_sb[:])
                    xs_sb = sbB.tile([N, H], F32, tag="xs_sb")
                    nc.vector.tensor_copy(xs_sb[:], xs_ps[:])

                    xr_ps = nps.tile([N, H], F32, tag="x_ps")
                    nc.tensor.matmul(
                        xr_ps[:], recvT[:, i * N:(i + 1) * N], wc1r_sb[:])
                    xr_sb = sbB.tile([N, H], F32, tag="xr_sb")
                    nc.vector.tensor_copy(xr_sb[:], xr_ps[:])

                    for ec in range(NEC):
                        pre = pps.tile([FC, NFC, EC], F32, tag="pre")
                        for fc in range(NFC):
                            nc.tensor.matmul(
                                pre[:, fc, :],
                                xs_sb[:, fc * FC:(fc + 1) * FC],
                                gs_sb[:, ec * EC:(ec + 1) * EC],
                                start=True, stop=False)
                            nc.tensor.matmul(
                                pre[:, fc, :],
                                xr_sb[:, fc * FC:(fc + 1) * FC],
                                gr_sb[:, ec * EC:(ec + 1) * EC],
                                start=False, stop=True)
                        hT = sbB.tile([FC, NFC, EC], F32, tag="hT")
                        for fc in range(NFC):
                            nc.scalar.activation(
                                hT[:, fc, :], pre[:, fc, :], relu,
                                bias=pet3_sb[:, fc * TCHUNK + i:
                                             fc * TCHUNK + i + 1])
                        o_ps = ops.tile([5, EC], F32, tag="x_ps")
                        for fc in range(NFC):
                            nc.tensor.matmul(o_ps[:], wc2_sb[:, fc, :],
                                             hT[:, fc, :],
                                             start=(fc == 0), stop=False)
                        nc.tensor.matmul(o_ps[:], bc2r_sb[:], ones_sb[:],
                                         start=False, stop=True)
                        o_sb = sbB.tile([5, EC], F32, tag="o_sb")
                        nc.vector.tensor_copy(o_sb[:], o_ps[:])
                        nc.sync.dma_start(
                            outd[i, :, ec * EC:(ec + 1) * EC], o_sb[:])

    _legalize_waits(nc)
    return nc


def _legalize_waits(nc):
    """Walrus codegen rejects instructions carrying more than one sync wait
    ("Too many sync wait commands", CoreV3GenImpl setupSyncWait). Hoist all
    but the last wait of any instruction onto standalone InstEventSemaphore
    instructions inserted just before it on the same engine queue —
    semantically identical, since waits execute in program order."""
    for f in nc.m.functions:
        for blk in f.blocks:
            insts = blk.instructions
            if not any(
                i.sync_info is not None and len(i.sync_info.on_wait or ()) > 1
                for i in insts
            ):
                continue
            out = []
            for inst in insts:
                si = inst.sync_info
                waits = list(si.on_wait) if si is not None and si.on_wait else []
                if len(waits) > 1:
                    for w in waits[:-1]:
                        out.append(mybir.InstEventSemaphore(
                            name=nc.get_next_instruction_name(),
                            engine=inst.engine,
                            ins=[],
                            outs=[],
                            sync_info=mybir.SyncInfo(on_wait=[w], on_update=[]),
                        ))
                    si.on_wait = waits[-1:]
                out.append(inst)
            blk.instructions = out


def _get_program():
    global _PROGRAM
    if _PROGRAM is None:
        _PROGRAM = _build_program()
    return _PROGRAM


def _sinusoidal_pe(d, t):
    pos = np.arange(t, dtype=np.float32)[:, None]
    div = np.exp(np.arange(0, d, 2, dtype=np.float32)
                 * (-np.log(10000.0) / d)).astype(np.float32)
    pe = np.zeros((t, d), dtype=np.float32)
    pe[:, 0::2] = np.sin(pos * div)
    pe[:, 1::2] = np.cos(pos * div)
    return pe


def kernel(spikes, W_emb, b_emb, Ws1, bs1, Ws2, bs2, Wr1, br1, Wr2, br2,
           Wc1, bc1, Wc2, bc2, send_edges, recv_edges):
    global LAST_RESULTS
    f32 = np.float32
    spikes = np.asarray(spikes, f32)
    W_emb = np.ascontiguousarray(np.asarray(W_emb, f32))
    Wc1 = np.asarray(Wc1, f32)
    Wc2 = np.asarray(Wc2, f32)
    se = np.asarray(send_edges).astype(np.int64)
    re_ = np.asarray(recv_edges).astype(np.int64)

    # Positional-encoding contribution to the pre-ReLU combine activations:
    # pet_full[t_out] = pe[t_out+1] @ Wc1[128:144] + pe[t_out] @ Wc1[272:288]
    #                   + bc1, shape [T-1, 288].
    pe = _sinusoidal_pe(PE_SIZE, T)
    pet_full = (pe[1:] @ Wc1[D:D + PE_SIZE]
                + pe[:-1] @ Wc1[D + PE_SIZE + D:]
                + np.asarray(bc1, f32)[None, :]).astype(f32)

    nodes = np.arange(N, dtype=np.int64)
    G_send = (se[None, :] == nodes[:, None]).astype(f32)        # [N, E]
    G_recv = (re_[None, :] == nodes[:, None]).astype(f32)       # [N, E]

    common = dict(
        w_emb=W_emb,
        b_emb=np.ascontiguousarray(np.asarray(b_emb, f32).reshape(1, D)),
        ws1=np.ascontiguousarray(np.asarray(Ws1, f32)),
        bs1=np.ascontiguousarray(np.asarray(bs1, f32).reshape(D, 1)),
        ws2=np.ascontiguousarray(np.asarray(Ws2, f32)),
        bs2=np.ascontiguousarray(np.asarray(bs2, f32).reshape(D, 1)),
        wr1=np.ascontiguousarray(np.asarray(Wr1, f32)),
        br1=np.ascontiguousarray(np.asarray(br1, f32).reshape(D, 1)),
        wr2=np.ascontiguousarray(np.asarray(Wr2, f32)),
        br2=np.ascontiguousarray(np.asarray(br2, f32).reshape(D, 1)),
        wc1s=np.ascontiguousarray(Wc1[0:D]),
        wc1r=np.ascontiguousarray(Wc1[D + PE_SIZE:D + PE_SIZE + D]),
        gs=G_send,
        gr=G_recv,
        wc2=np.ascontiguousarray(Wc2.reshape(NFC, FC, 5).transpose(1, 0, 2)),
        bc2r=np.ascontiguousarray(np.asarray(bc2, f32).reshape(1, 5)),
        ones=np.ones((1, EC), f32),
    )

    in_maps = []
    for core in range(NCORES):
        b = core // 4
        t_lo = T_LOS[core % 4]
        spk_slice = spikes[b, t_lo:t_lo + TSTEPS]               # [65,128,16]
        spkT = np.ascontiguousarray(
            spk_slice.reshape(ROWS, F).T)                       # [16, 8320]
        pet = pet_full[t_lo:t_lo + TCHUNK]                      # [64, 288]
        pet3 = np.ascontiguousarray(
            pet.reshape(TCHUNK, NFC, FC).transpose(2, 1, 0)
            .reshape(FC, NFC * TCHUNK))                         # [96, 192]
        in_maps.append(dict(common, spk=spkT, pet3=pet3))

    nc = _get_program()
    trace = bool(int(os.environ.get("KERNEL_TRACE", "0")))
    res = run_bass_kernel_spmd(nc, in_maps, list(range(NCORES)), trace=trace)
    LAST_RESULTS = res

    out = np.zeros((B, T - 1, E, 5), f32)
    for core in range(NCORES):
        b = core // 4
        t_lo = T_LOS[core % 4]
        r = res.results[core]["out"]                            # [64, 5, 1024]
        out[b, t_lo:t_lo + TCHUNK] = r.transpose(0, 2, 1)
    return out


# revision 5
# speedup vs baseline: 2.1190x; 2.1190x over previous
"""Trainium2 Bass kernel for nn_Encoder_85942295593405 (GNN message passing).

Math (reference):
  emb  = spikes @ W_emb + b_emb                      [b,t,N,D]
  send = relu(relu(emb@Ws1+bs1)@Ws2+bs2)             [b,t,N,D]
  recv = relu(relu(emb@Wr1+br1)@Wr2+br2)             [b,t,N,D]
  full = [send[:,1:,se]|pe[1:]|recv[:,:-1,re]|pe[:-1]]   [b,t-1,E,288]
  out  = relu(full@Wc1+bc1)@Wc2 + bc2                [b,t-1,E,5]

Key factorization: the edge gather commutes with the (linear) first combine
layer, so compute Xs = send@Wc1[0:128,:], Xr = recv@Wc1[144:272,:] at NODE
level (N=128 rows instead of E=1024), then gather to edges via one-hot
gather-matmuls accumulated in PSUM (gather + add fused on the PE), apply the
t-dependent positional-encoding contribution as a per-partition ACT bias
fused with the ReLU, and finish with the thin @Wc2 matmul (bc2 folded in as
a K=1 rank-1 matmul against a ones row).

Sharding: 8 cores = 2 batches x 4 time chunks. Each core computes 64
output timesteps (chunk starts [0,64,128,191]; the last chunk overlaps the
third by one step so all cores run an identical program).
"""

import os
import sys

import numpy as np

sys.path.insert(0, "/opt/trn_rl_repo")

import concourse.bass as bass  # noqa: E402
import concourse.mybir as mybir  # noqa: E402
import concourse.tile as tile  # noqa: E402
from concourse.bass_utils import run_bass_kernel_spmd  # noqa: E402

B, T, N, F = 2, 256, 128, 16
D, H, E = 128, 288, 1024
PE_SIZE = 16
NCORES = 8
TCHUNK = 64            # output timesteps per core
TSTEPS = TCHUNK + 1    # node-level timesteps per core
ROWS = TSTEPS * N      # node-level rows per core (8320)
T_LOS = [0, 64, 128, 191]
FC = 96                # feature chunk for the 288-wide edge activations
NFC = 3                # number of feature chunks (3*96 = 288)
EC = 512               # edge chunk (moving-operand max for fp32)
NEC = E // EC

F32 = mybir.dt.float32
BF16 = mybir.dt.bfloat16

LAST_RESULTS = None    # BassKernelResults of the last run (for test harness)

_PROGRAM = None


def _build_program():
    nc = bass.Bass()

    def inp(name, shape):
        return nc.dram_tensor(name, shape, F32, kind="ExternalInput")

    spk = inp("spk", [F, ROWS])          # spikes, feature-major, per-core slice
    w_emb = inp("w_emb", [F, D])
    ws1 = inp("ws1", [D, D])
    bs1 = inp("bs1", [D, 1])             # column (per-partition ACT bias)
    ws2 = inp("ws2", [D, D])
    bs2 = inp("bs2", [D, 1])
    wr1 = inp("wr1", [D, D])
    br1 = inp("br1", [D, 1])
    wr2 = inp("wr2", [D, D])
    br2 = inp("br2", [D, 1])
    wc1s = inp("wc1s", [D, H])           # Wc1[0:128, :]
    wc1r = inp("wc1r", [D, H])           # Wc1[144:272, :]
    gs = nc.dram_tensor("gs", [N, E], BF16, kind="ExternalInput")   # one-hot
    gr = nc.dram_tensor("gr", [N, E], BF16, kind="ExternalInput")   # one-hot
    pet3 = inp("pet3", [FC, NFC * TCHUNK])  # pe@Wc1 slices + bc1, [96, fc*64+i]
    wc2 = nc.dram_tensor("wc2", [FC, NFC, 5], BF16, kind="ExternalInput")

    outd = nc.dram_tensor("out", [TCHUNK, 5, E], F32, kind="ExternalOutput")

    relu = mybir.ActivationFunctionType.Relu

    with tile.TileContext(nc) as tc:
        with tc.tile_pool(name="wpool", bufs=1) as wp:
            # persistent weights/constants in SBUF
            w_emb_sb = wp.tile([F, D], F32, tag="w_emb")
            ws1_sb = wp.tile([D, D], F32, tag="ws1")
            bs1_sb = wp.tile([D, 1], F32, tag="bs1")
            ws2_sb = wp.tile([D, D], F32, tag="ws2")
            bs2_sb = wp.tile([D, 1], F32, tag="bs2")
            wr1_sb = wp.tile([D, D], F32, tag="wr1")
            br1_sb = wp.tile([D, 1], F32, tag="br1")
            wr2_sb = wp.tile([D, D], F32, tag="wr2")
            br2_sb = wp.tile([D, 1], F32, tag="br2")
            wc1s_sb = wp.tile([D, H], F32, tag="wc1s")
            wc1r_sb = wp.tile([D, H], F32, tag="wc1r")
            gs_sb = wp.tile([N, E], BF16, tag="gs")
            gr_sb = wp.tile([N, E], BF16, tag="gr")
            pet3_sb = wp.tile([FC, NFC * TCHUNK], F32, tag="pet3")
            wc2_sb = wp.tile([FC, NFC, 5], BF16, tag="wc2")
            sendT = wp.tile([D, ROWS], F32, tag="sendT")   # node tables,
            recvT = wp.tile([D, ROWS], F32, tag="recvT")   # feature-major

            for sb_t, dr_t in [
                (w_emb_sb, w_emb),
                (ws1_sb, ws1), (bs1_sb, bs1), (ws2_sb, ws2), (bs2_sb, bs2),
                (wr1_sb, wr1), (br1_sb, br1), (wr2_sb, wr2), (br2_sb, br2),
                (wc1s_sb, wc1s), (wc1r_sb, wc1r), (gs_sb, gs), (gr_sb, gr),
                (pet3_sb, pet3), (wc2_sb, wc2),
            ]:
                nc.sync.dma_start(sb_t[:], dr_t[:])

            # ---- Stage A: node-level MLPs, feature-major ----
            # sendT/recvT[f, k*128+n] = send/recv branch value at local step k,
            # node n, feature f.
            chunks = []
            r0 = 0
            while r0 < ROWS:
                ch = min(512, ROWS - r0)
                chunks.append((r0, ch))
                r0 += ch

            with (
                tc.tile_pool(name="stA_ps", bufs=1, space="PSUM") as psA,
                tc.tile_pool(name="stA_sb", bufs=3) as sbA,
            ):
                for r0, ch in chunks:
                    spk_c = sbA.tile([F, ch], F32, tag="spk")
                    nc.sync.dma_start(spk_c[:], spk[:, r0:r0 + ch])

                    emb_ps = psA.tile([D, ch], F32, tag="emb_ps")
                    nc.tensor.matmul(emb_ps[:], w_emb_sb[:], spk_c[:])
                    emb_sb = sbA.tile([D, ch], F32, tag="emb_sb")
                    nc.vector.tensor_copy(emb_sb[:], emb_ps[:])

                    # send branch
                    s1_ps = psA.tile([D, ch], F32, tag="s1_ps")
                    nc.tensor.matmul(s1_ps[:], ws1_sb[:], emb_sb[:])
                    s1_sb = sbA.tile([D, ch], F32, tag="s1_sb")
                    nc.scalar.activation(s1_sb[:], s1_ps[:], relu,
                                         bias=bs1_sb[:, 0:1])
                    s2_ps = psA.tile([D, ch], F32, tag="s2_ps")
                    nc.tensor.matmul(s2_ps[:], ws2_sb[:], s1_sb[:])
                    nc.scalar.activation(sendT[:, r0:r0 + ch], s2_ps[:], relu,
                                         bias=bs2_sb[:, 0:1])

                    # recv branch
                    r1_ps = psA.tile([D, ch], F32, tag="r1_ps")
                    nc.tensor.matmul(r1_ps[:], wr1_sb[:], emb_sb[:])
                    r1_sb = sbA.tile([D, ch], F32, tag="r1_sb")
                    nc.scalar.activation(r1_sb[:], r1_ps[:], relu,
                                         bias=br1_sb[:, 0:1])
                    r2_ps = psA.tile([D, ch], F32, tag="r2_ps")
                    nc.tensor.matmul(r2_ps[:], wr2_sb[:], r1_sb[:])
                    nc.scalar.activation(recvT[:, r0:r0 + ch], r2_ps[:], relu,
                                         bias=br2_sb[:, 0:1])

            # ---- Stage B: per output timestep ----
            with (
                tc.tile_pool(name="node_ps", bufs=2, space="PSUM") as nps,
                tc.tile_pool(name="pre_ps", bufs=2, space="PSUM") as pps,
                tc.tile_pool(name="stB_sb", bufs=3) as sbB,
            ):
                ops = nps  # share the 1-bank slots: xs/xr/o_ps all tag "x_ps"
                for i in range(TCHUNK):
                    # Xs at local step i+1 (sender at t+1), Xr at step i.
                    xs_ps = nps.tile([N, H], F32, tag="x_ps")
                    nc.tensor.matmul(
                        xs_ps[:], sendT[:, (i + 1) * N:(i + 2) * N], wc1s_sb[:])
                    xs_sb = sbB.tile([N, H], BF16, tag="xs_sb")
                    nc.vector.tensor_copy(xs_sb[:], xs_ps[:])

                    xr_ps = nps.tile([N, H], F32, tag="x_ps")
                    nc.tensor.matmul(
                        xr_ps[:], recvT[:, i * N:(i + 1) * N], wc1r_sb[:])
                    xr_sb = sbB.tile([N, H], BF16, tag="xr_sb")
                    nc.vector.tensor_copy(xr_sb[:], xr_ps[:])

                    for ec in range(NEC):
                        pre = pps.tile([FC, NFC, EC], F32, tag="pre")
                        for fc in range(NFC):
                            nc.tensor.matmul(
                                pre[:, fc, :],
                                xs_sb[:, fc * FC:(fc + 1) * FC],
                                gs_sb[:, ec * EC:(ec + 1) * EC],
                                start=True, stop=False)
                            nc.tensor.matmul(
                                pre[:, fc, :],
                                xr_sb[:, fc * FC:(fc + 1) * FC],
                                gr_sb[:, ec * EC:(ec + 1) * EC],
                                start=False, stop=True)
                        hT = sbB.tile([FC, NFC, EC], BF16, tag="hT")
                        for fc in range(NFC):
                            nc.scalar.activation(
                                hT[:, fc, :], pre[:, fc, :], relu,
                                bias=pet3_sb[:, fc * TCHUNK + i:
                                             fc * TCHUNK + i + 1])
                        o_ps = ops.tile([5, EC], F32, tag="x_ps")
                        for fc in range(NFC):
                            nc.tensor.matmul(o_ps[:], wc2_sb[:, fc, :],
                                             hT[:, fc, :],
                                             start=(fc == 0), stop=(fc == NFC - 1))
                        o_sb = sbB.tile([5, EC], F32, tag="o_sb")
                        nc.vector.tensor_copy(o_sb[:], o_ps[:])
                        nc.sync.dma_start(
                            outd[i, :, ec * EC:(ec + 1) * EC], o_sb[:])

    _legalize_waits(nc)
    return nc


def _legalize_waits(nc):
    """Walrus codegen rejects instructions carrying more than one sync wait
    ("Too many sync wait commands", CoreV3GenImpl setupSyncWait). Hoist all
    but the last wait of any instruction onto standalone InstEventSemaphore
    instructions inserted just before it on the same engine queue —
    semantically identical, since waits execute in program order."""
    for f in nc.m.functions:
        for blk in f.blocks:
            insts = blk.instructions
            if not any(
                i.sync_info is not None and len(i.sync_info.on_wait or ()) > 1
                for i in insts
            ):
                continue
            out = []
            for inst in insts:
                si = inst.sync_info
                waits = list(si.on_wait) if si is not None and si.on_wait else []
                if len(waits) > 1:
                    for w in waits[:-1]:
                        out.append(mybir.InstEventSemaphore(
                            name=nc.get_next_instruction_name(),
                            engine=inst.engine,
                            ins=[],
                            outs=[],
                            sync_info=mybir.SyncInfo(on_wait=[w], on_update=[]),
                        ))
                    si.on_wait = waits[-1:]
                out.append(inst)
            blk.instructions = out


def _get_program():
    global _PROGRAM
    if _PROGRAM is None:
        _PROGRAM = _build_program()
    return _PROGRAM


def _sinusoidal_pe(d, t):
    pos = np.arange(t, dtype=np.float32)[:, None]
    div = np.exp(np.arange(0, d, 2, dtype=np.float32)
                 * (-np.log(10000.0) / d)).astype(np.float32)
    pe = np.zeros((t, d), dtype=np.float32)
    pe[:, 0::2] = np.sin(pos * div)
    pe[:, 1::2] = np.cos(pos * div)
    return pe


def kernel(spikes, W_emb, b_emb, Ws1, bs1, Ws2, bs2, Wr1, br1, Wr2, br2,
           Wc1, bc1, Wc2, bc2, send_edges, recv_edges):
    global LAST_RESULTS
    f32 = np.float32
    spikes = np.asarray(spikes, f32)
    W_emb = np.ascontiguousarray(np.asarray(W_emb, f32))
    Wc1 = np.asarray(Wc1, f32)
    Wc2 = np.asarray(Wc2, f32)
    se = np.asarray(send_edges).astype(np.int64)
    re_ = np.asarray(recv_edges).astype(np.int64)

    # Positional-encoding contribution to the pre-ReLU combine activations:
    # pet_full[t_out] = pe[t_out+1] @ Wc1[128:144] + pe[t_out] @ Wc1[272:288]
    #                   + bc1, shape [T-1, 288].
    pe = _sinusoidal_pe(PE_SIZE, T)
    pet_full = (pe[1:] @ Wc1[D:D + PE_SIZE]
                + pe[:-1] @ Wc1[D + PE_SIZE + D:]
                + np.asarray(bc1, f32)[None, :]).astype(f32)

    nodes = np.arange(N, dtype=np.int64)
    G_send = (se[None, :] == nodes[:, None]).astype(f32)        # [N, E]
    G_recv = (re_[None, :] == nodes[:, None]).astype(f32)       # [N, E]

    import ml_dtypes
    bf16 = ml_dtypes.bfloat16
    # fold the (activation-free) embed bias into the first MLP-layer biases:
    # (emb + b_emb) @ W + b == emb @ W + (b + b_emb @ W)
    b_emb_v = np.asarray(b_emb, f32).reshape(1, D)
    bs1_f = np.asarray(bs1, f32) + (b_emb_v @ np.asarray(Ws1, f32))[0]
    br1_f = np.asarray(br1, f32) + (b_emb_v @ np.asarray(Wr1, f32))[0]
    common = dict(
        w_emb=W_emb,
        ws1=np.ascontiguousarray(np.asarray(Ws1, f32)),
        bs1=np.ascontiguousarray(bs1_f.reshape(D, 1)),
        ws2=np.ascontiguousarray(np.asarray(Ws2, f32)),
        bs2=np.ascontiguousarray(np.asarray(bs2, f32).reshape(D, 1)),
        wr1=np.ascontiguousarray(np.asarray(Wr1, f32)),
        br1=np.ascontiguousarray(br1_f.reshape(D, 1)),
        wr2=np.ascontiguousarray(np.asarray(Wr2, f32)),
        br2=np.ascontiguousarray(np.asarray(br2, f32).reshape(D, 1)),
        wc1s=np.ascontiguousarray(Wc1[0:D]),
        wc1r=np.ascontiguousarray(Wc1[D + PE_SIZE:D + PE_SIZE + D]),
        gs=G_send.astype(bf16),
        gr=G_recv.astype(bf16),
        wc2=np.ascontiguousarray(
            Wc2.reshape(NFC, FC, 5).transpose(1, 0, 2)).astype(bf16),
    )

    in_maps = []
    for core in range(NCORES):
        b = core // 4
        t_lo = T_LOS[core % 4]
        spk_slice = spikes[b, t_lo:t_lo + TSTEPS]               # [65,128,16]
        spkT = np.ascontiguousarray(
            spk_slice.reshape(ROWS, F).T)                       # [16, 8320]
        pet = pet_full[t_lo:t_lo + TCHUNK]                      # [64, 288]
        pet3 = np.ascontiguousarray(
            pet.reshape(TCHUNK, NFC, FC).transpose(2, 1, 0)
            .reshape(FC, NFC * TCHUNK))                         # [96, 192]
        in_maps.append(dict(common, spk=spkT, pet3=pet3))

    nc = _get_program()
    trace = bool(int(os.environ.get("KERNEL_TRACE", "0")))
    res = run_bass_kernel_spmd(nc, in_maps, list(range(NCORES)), trace=trace)
    LAST_RESULTS = res

    out = np.zeros((B, T - 1, E, 5), f32)
    for core in range(NCORES):
        b = core // 4
        t_lo = T_LOS[core % 4]
        r = res.results[core]["out"]                            # [64, 5, 1024]
        out[b, t_lo:t_lo + TCHUNK] = r.transpose(0, 2, 1)
    out += np.asarray(bc2, f32)[None, None, None, :]
    return out
